# revision 1
# baseline (speedup 1.0000x reference)
"""Trainium2 Bass kernel for causal masked-ReLU attention (no softmax).

Reference computation (B=8, T=1024, C=768, n_head=12, hd=64):
    qkv = x @ W_attn.T + b_attn
    q, k, v = split(qkv); per-head: att = relu(mask_causal(q k^T / sqrt(hd)))
    y = att @ v, heads re-merged -> (B, T, C)

Sharding: one batch element per NeuronCore (8 cores). Each core computes the
QKV projection and all 12 heads' attention for its batch element.

Layout strategy (per core):
  - Host passes x[b].T (C, T) and W.T (C, 3C) so the contraction dim C lands
    on SBUF partitions with unit-stride DMA (no on-chip transposes).
  - W rows are pre-permuted on host into [q-pair0, k-pair0, q-pair1, ...] so
    q.T / k.T of head h live at the same partition offset (h%2)*64 of their
    M-tiles; matmul operands then share a base partition.
  - q weights/bias are pre-scaled by 1/sqrt(hd) on host.
  - att is computed transposed (att.T = k @ q.T, layout [T_k, T_q]) so the AV
    matmul (y.T = v.T @ att.T) streams att.T directly with v as stationary.
  - Causal structure at 256-col granularity: fully-masked windows are never
    computed; att.T below-diagonal regions are zeroed once and never written.
  - All matmuls run as float32r (TF32-like, full PE rate at N>=256),
    fp32 storage, fp32 PSUM accumulation.
  - Inputs ship as fp16 (the PE truncates fp32 operands to ~10 mantissa
    bits anyway), halving the DMA ramp; PSUM accumulation stays fp32.
  - Input DMAs are chained (~6 in flight) in contraction-tile order and the
    projection runs k-outer over windows of 6 PSUM groups, so the PE starts
    as soon as the first k-tiles land instead of waiting for the full load.
  - QK evictions and AV accumulation groups both run ascending, so att
    tiles are read and released in the same order the next head rewrites
    them, and the QK eviction stream ends on the cheap diagonal tiles.
  - Eviction work (masked-ReLU, bias adds, y copies) is balanced across
    the ACT and DVE engines, which sit just under the PE's per-head time.
  - A few dummy matmuls during the initial DMA wait keep the PE's HAM
    activity window warm so real matmuls start at the full 2.4 GHz clock.
  - Output is written as y.T (C, T) in fp16; host transposes and upcasts.
"""

import numpy as np

import sys
for _p in ("/opt/trn_rl_repo", "/root/.axon_site", "/root/.axon_site/_ro/trn_rl_repo",
           "/root/.axon_site/_ro/pypackages"):
    if _p not in sys.path:
        sys.path.append(_p)

import concourse.bacc as bacc
import concourse.mybir as mybir
from concourse.alu_op_type import AluOpType
from concourse.tile import TileContext
from concourse.tile_rust import add_dep_helper
from concourse.bass_utils import run_bass_kernel_spmd

B, T, C = 8, 1024, 768
NH, HD = 12, 64
C3 = 3 * C            # 2304
KT = C // 128         # 6  contraction tiles of the projection
TT = T // 128         # 8  tiles of the sequence dim
NPAIR = NH // 2       # 6  head pairs
NW = T // 256         # 4  256-wide attention windows
F32 = mybir.dt.float32
F16 = mybir.dt.float16
AF = mybir.ActivationFunctionType

WARM_MMS = 8

_CACHE = {}


def _build():
    nc = bacc.Bacc("TRN2", target_bir_lowering=False, debug=False, num_devices=8)

    xT = nc.dram_tensor("xT", [C, T], F16, kind="ExternalInput").ap()
    wT = nc.dram_tensor("wT", [C, C3], F16, kind="ExternalInput").ap()
    bqk = nc.dram_tensor("bqk", [128, 2 * NPAIR], F32, kind="ExternalInput").ap()
    bvb = nc.dram_tensor("bvb", [128, C], F32, kind="ExternalInput").ap()
    # masks = [tri(128) | ones(896)]: the kept region of att.T tile tk always
    # starts with the triangular diagonal block, so masks[:, :width] is the
    # relu-mask for any kept window
    masks = nc.dram_tensor("masks", [128, T], F32, kind="ExternalInput").ap()
    zeros = nc.dram_tensor("zeros", [128, T - 128], F16, kind="ExternalInput").ap()
    yT = nc.dram_tensor("yT", [C, T], F16, kind="ExternalOutput").ap()

    dma_chain = []
    CHAIN_DEPTH = 6

    def chained_dma(out, in_):
        # keep ~CHAIN_DEPTH input DMAs in flight: enough to pipeline the DMA
        # queues, few enough that k-tiles still arrive roughly in issue order
        inst = nc.sync.dma_start(out=out, in_=in_)
        if len(dma_chain) >= CHAIN_DEPTH:
            add_dep_helper(inst.ins, dma_chain[-CHAIN_DEPTH].ins, True,
                           "dma staging chain")
        dma_chain.append(inst)
        return inst

    with TileContext(nc) as tc:
        with (
            tc.tile_pool(name="persist", bufs=1) as pp,
            tc.tile_pool(name="psum_y", bufs=2, space="PSUM") as ps_y,
        ):
            masks_sb = pp.tile([128, T], F32, name="masks_sb")
            bqk_sb = pp.tile([128, 2 * NPAIR], F32, name="bqk_sb")
            bvb_sb = pp.tile([128, C], F32, name="bvb_sb")
            qkT = [pp.tile([128, T], F16, name=f"qkT{m}") for m in range(2 * NPAIR)]
            v_sb = [pp.tile([128, C], F16, name=f"v{t}") for t in range(TT)]
            attsets = [[pp.tile([128, T], F16, name=f"att{s}_{t}")
                        for t in range(TT)] for s in range(2)]

            # ---------- Phase 1: QKV projection ----------
            with (
                tc.tile_pool(name="io", bufs=1) as iop,
                tc.tile_pool(name="psum_proj", bufs=6, space="PSUM") as ps_proj,
            ):
                w_sb = [iop.tile([128, C3], F16, name=f"w{k}") for k in range(KT)]
                x_sb = [iop.tile([128, T], F16, name=f"x{k}") for k in range(KT)]

                # input DMAs, staged so k-tiles arrive in order
                smalls = [(bvb_sb, bvb), (bqk_sb, bqk)]
                for k in range(KT):
                    chained_dma(x_sb[k][:], xT[128 * k:128 * (k + 1), :])
                    chained_dma(w_sb[k][:, 2 * C:],
                                wT[128 * k:128 * (k + 1), 2 * C:])
                    if smalls and k >= 1:
                        dst, src_ = smalls.pop(0)
                        chained_dma(dst[:], src_[:])
                for k in range(KT):
                    chained_dma(w_sb[k][:, :2 * C],
                                wT[128 * k:128 * (k + 1), :2 * C])
                chained_dma(masks_sb[:], masks[:])
                # below-diagonal regions of att stay zero for the whole
                # kernel; evictions only ever write cols >= 128*t
                for s in range(2):
                    for t in range(1, TT):
                        chained_dma(attsets[s][t][:, :128 * t],
                                    zeros[:, :128 * t])

                # PE warmup: dummy matmuls on a never-written scratch tile
                # during the initial DMA wait; keeps the HAM activity window
                # busy so the real matmuls start at full clock. Results (and
                # operand garbage) are discarded.
                scratch = iop.tile([128, 512], F16, name="warm_src")
                nc.vector.memset(scratch[:], 0.0)
                warm = ps_proj.tile([128, 512], F32, name="ps_warm", tag="ps_proj")
                for _ in range(WARM_MMS):
                    nc.tensor.matmul(warm[:], scratch[:, :128], scratch[:],
                                     start=True, stop=True)

                # v groups first: their DMAs land first
                groups = []
                for t in range(TT):
                    for n0, nw_ in ((0, 512), (512, 256)):
                        groups.append(("v", t, n0, nw_))
                for m in range(2 * NPAIR):
                    for n in range(2):
                        groups.append(("qk", m, 512 * n, 512))

                # windows of 6 psum groups, k-outer within the window so the
                # PE starts on k-tiles as they land
                for w0 in range(0, len(groups), 5):
                    window = groups[w0:w0 + 5]
                    tiles = [ps_proj.tile([128, g[3]], F32, name="ps_proj",
                                          tag="ps_proj") for g in window]
                    for k in range(KT):
                        for g, ps in zip(window, tiles):
                            if g[0] == "v":
                                _, t, n0, nw_ = g
                                nc.tensor.matmul(
                                    ps[:],
                                    x_sb[k][:, 128 * t:128 * (t + 1)],
                                    w_sb[k][:, 2 * C + n0:2 * C + n0 + nw_],
                                    start=(k == 0), stop=(k == KT - 1),
                                )
                            else:
                                _, m, q0, nw_ = g
                                nc.tensor.matmul(
                                    ps[:],
                                    w_sb[k][:, 128 * m:128 * (m + 1)],
                                    x_sb[k][:, q0:q0 + nw_],
                                    start=(k == 0), stop=(k == KT - 1),
                                )
                    for g, ps in zip(window, tiles):
                        if g[0] == "v":
                            _, t, n0, nw_ = g
                            nc.vector.tensor_tensor(
                                v_sb[t][:, n0:n0 + nw_], ps[:],
                                bvb_sb[:, n0:n0 + nw_], AluOpType.add,
                            )
                        else:
                            _, m, q0, nw_ = g
                            if m % 2 == 0:
                                nc.scalar.activation(
                                    qkT[m][:, q0:q0 + nw_], ps[:],
                                    AF.Identity, bias=bqk_sb[:, m:m + 1], scale=1.0,
                                )
                            else:
                                nc.vector.tensor_scalar(
                                    qkT[m][:, q0:q0 + nw_], ps[:],
                                    bqk_sb[:, m:m + 1], None,
                                    AluOpType.add,
                                )

            # ---------- Phase 2: attention, head by head ----------
            with (
                tc.tile_pool(name="psum_att", bufs=6, space="PSUM") as ps_att,
                tc.tile_pool(name="yout", bufs=2) as yop,
            ):
                for a in range(NPAIR):
                    y_pair = yop.tile([128, T], F16, name="y_pair", tag="y_pair")
                    for phase in range(2):   # 0 = QK both heads, 1 = AV both heads
                      for r in range(2):
                        h = 2 * a + r
                        att = attsets[r]
                        qh = qkT[2 * a][64 * r:64 * (r + 1), :]
                        kh = qkT[2 * a + 1][64 * r:64 * (r + 1), :]
                        if phase == 0:
                          # QK^T -> att.T, tk ascending, single-bank psum tiles
                          # for fine-grained slot recycling. Ascending order
                          # ends on the cheap evictions (tk6/7 have no relu),
                          # so AV never waits behind an eviction backlog.
                          for tk in range(TT):
                              k0 = 128 * tk
                              q0d = 256 * (tk // 2)       # start of diag window
                              # piece 1: [k0, 512) if the diag sits left of 512
                              if q0d < 512:
                                  tag = "ps_y" if (h == 0 and tk <= 1) else "ps_att"
                                  pool = ps_y if tag == "ps_y" else ps_att
                                  ps = pool.tile([128, 512], F32,
                                                 name="ps_qk", tag=tag)
                                  pw = 512 - q0d
                                  nc.tensor.matmul(
                                      ps[:, k0 - q0d:pw], kh[:, k0:k0 + 128],
                                      qh[:, k0:512], start=True, stop=True,
                                  )
                                  # whole piece in one DVE op: relu * [tri|1..]
                                  nc.vector.scalar_tensor_tensor(
                                      att[tk][:, k0:512],
                                      ps[:, k0 - q0d:pw],
                                      0.0, masks_sb[:, :512 - k0],
                                      AluOpType.max, AluOpType.mult,
                                  )
                                  # piece 2: the full [512, 1024) half
                                  ps = pool.tile([128, 512], F32,
                                                 name="ps_qk", tag=tag)
                                  nc.tensor.matmul(
                                      ps[:], kh[:, k0:k0 + 128], qh[:, 512:T],
                                      start=True, stop=True,
                                  )
                                  nc.scalar.activation(att[tk][:, 512:T], ps[:],
                                                       AF.Relu)
                              else:
                                  # single piece [k0, 1024)
                                  ps = ps_att.tile([128, 512], F32,
                                                   name="ps_qk", tag="ps_att")
                                  pw = T - q0d
                                  nc.tensor.matmul(
                                      ps[:, k0 - q0d:pw], kh[:, k0:k0 + 128],
                                      qh[:, k0:T], start=True, stop=True,
                                  )
                                  nc.vector.scalar_tensor_tensor(
                                      att[tk][:, k0:T],
                                      ps[:, k0 - q0d:pw],
                                      0.0, masks_sb[:, :T - k0],
                                      AluOpType.max, AluOpType.mult,
                                  )

                        if phase == 1:
                          # AV: y.T = v.T @ att.T, groups j ascending, paired
                          # into one [64, 512] psum tile per half; att tiles are
                          # read and released in the order the next head's QK
                          # rewrites them
                          jhis = (3, 1) if h == NH - 1 else (1, 3)
                          for jhi in jhis:
                              ps2 = ps_y.tile([64, 512], F32, name="ps_av",
                                              tag="ps_y")
                              for j in (jhi - 1, jhi):
                                  q0 = 256 * j
                                  c0 = 256 * (j - (jhi - 1))
                                  ntk = min(TT, 2 * j + 2)
                                  for tk in range(ntk):
                                      lo = 128 if tk == 2 * j + 1 else 0
                                      nc.tensor.matmul(
                                          ps2[:, c0 + lo:c0 + 256],
                                          v_sb[tk][:, 64 * h:64 * (h + 1)],
                                          att[tk][:, q0 + lo:q0 + 256],
                                          start=(tk == 0), stop=(tk == ntk - 1),
                                      )
                              nc.scalar.copy(
                                  y_pair[64 * r:64 * (r + 1),
                                         256 * (jhi - 1):256 * (jhi + 1)],
                                  ps2[:],
                              )
                              nc.sync.dma_start(
                                  out=yT[128 * a + 64 * r:128 * a + 64 * (r + 1),
                                         256 * (jhi - 1):256 * (jhi + 1)],
                                  in_=y_pair[64 * r:64 * (r + 1),
                                             256 * (jhi - 1):256 * (jhi + 1)])

    nc.compile()
    return nc

def _prep_host(x, W_attn, b_attn):
    s = 1.0 / np.sqrt(np.float32(HD))
    W = np.asarray(W_attn, dtype=np.float32).copy()
    b = np.asarray(b_attn, dtype=np.float32).copy()
    W[:C] *= s
    b[:C] *= s
    # interleave q/k head pairs: [q-pair0, k-pair0, q-pair1, k-pair1, ...], v natural
    rows = []
    for a in range(NPAIR):
        rows.extend(range(128 * a, 128 * (a + 1)))          # q heads 2a, 2a+1
        rows.extend(range(C + 128 * a, C + 128 * (a + 1)))  # k heads 2a, 2a+1
    rows.extend(range(2 * C, 3 * C))                        # v natural
    W_perm = W[rows]
    b_perm = b[rows]

    wT = np.ascontiguousarray(W_perm.T.astype(np.float16))   # (C, 3C)
    bqk = np.ascontiguousarray(b_perm[:2 * C].reshape(2 * NPAIR, 128).T)  # (128, 12)
    bvb = np.ascontiguousarray(np.broadcast_to(b_perm[2 * C:], (128, C)))
    tri = (np.arange(128)[None, :] >= np.arange(128)[:, None]).astype(np.float32)
    masks = np.ones((128, T), dtype=np.float32)
    masks[:, 0:128] = tri          # kept windows always start at the diagonal
    zeros = np.zeros((128, T - 128), dtype=np.float16)
    xT = np.ascontiguousarray(np.asarray(x, dtype=np.float32).transpose(0, 2, 1).astype(np.float16))  # (B, C, T)
    return xT, wT, bqk, bvb, masks, zeros


def kernel(x, W_attn, b_attn):
    if "nc" not in _CACHE:
        _CACHE["nc"] = _build()
    nc = _CACHE["nc"]

    xT, wT, bqk, bvb, masks, zeros = _prep_host(x, W_attn, b_attn)
    in_maps = [
        {"xT": xT[c], "wT": wT, "bqk": bqk, "bvb": bvb, "masks": masks,
         "zeros": zeros}
        for c in range(B)
    ]
    res = run_bass_kernel_spmd(nc, in_maps, list(range(B)))
    y = np.empty((B, T, C), dtype=np.float32)
    for c in range(B):
        y[c] = res.results[c]["yT"].T.astype(np.float32)
    return y

